# revision 11
# baseline (speedup 1.0000x reference)
"""Causal self-attention (B=4, T=2048, C=1024, H=16, Dh=64) on 8 TRN2 cores.

Sharding: tensor-parallel over heads. Core c owns heads {2c, 2c+1}:
  - w_qkv column-sliced -> [1024, 384] (128 q-cols | 128 k-cols | 128 v-cols)
  - w_proj row-sliced   -> [128, 1024]
  - x is broadcast to all cores, pre-transposed on host to feature-major
    x_t [1024, B*T] so the contraction dim (C) lands on SBUF partitions.
  - each core emits a partial y [B*T, 1024]; host sums the 8 partials and
    adds b_proj.

Per-core pipeline (all matmuls in fp32r = full-rate fp32):
  A. QKV: qkv_T = w_slice.T @ x_t  -> q,k feature-major [128, T]; v likewise.
  B. v transposed to token-major tiles with a fused ones-column [Tk, 65]
     (column 64 of each att@v output row block = softmax denominator).
  C. Attention per (head, 1024-query group): S^T tile per 128-key chunk =
     k_tile.T @ q (K=64 matmul), transposed-causal mask add on the diagonal
     block, Exp on ACT (scale=1/8, no max subtraction - scores are O(10)),
     att@v accumulates y^T[65, Tq] over key chunks in PSUM. Softmax
     normalization is applied after att@v: r = 1/l broadcast across
     partitions with a K=1 ones matmul, one DVE multiply.
  D. Proj: y_part[tok, :] = y_T.T @ w_proj_slice, DMA'd out token-major.
"""

import sys

sys.path.insert(0, "/opt/trn_rl_repo")
import numpy as np

B = 4
T = 2048
C = 1024
H = 16
DH = 64
NCORES = 8
CPC = 128  # channels (=2 heads * 64) per core
TT = B * T
P = 128

TRACE = False
LAST_EXEC_NS = None

import os as _os

F_BIAS_DVE = _os.environ.get("F_BIAS_DVE", "1") == "1"
F_VAUG_MERGE = _os.environ.get("F_VAUG_MERGE", "1") == "1"
F_EPI = int(_os.environ.get("F_EPI", "1"))

_NC_CACHE = []


def _build():
    import concourse.bacc as bacc
    import concourse.mybir as mybir
    import concourse.tile as tile
    from concourse.masks import make_identity

    f32 = mybir.dt.float32
    r32 = mybir.dt.float32r
    Identity = mybir.ActivationFunctionType.Identity
    Exp = mybir.ActivationFunctionType.Exp
    ADD = mybir.AluOpType.add
    MULT = mybir.AluOpType.mult

    nc = bacc.Bacc(target_bir_lowering=False)

    x_t = nc.dram_tensor("x_t", [C, TT], f32, kind="ExternalInput")
    w_qkv_c = nc.dram_tensor("w_qkv_c", [C, 3 * CPC], f32, kind="ExternalInput")
    b_qkv_c = nc.dram_tensor("b_qkv_c", [P, 3], f32, kind="ExternalInput")
    w_proj_c = nc.dram_tensor("w_proj_c", [CPC, C], f32, kind="ExternalInput")
    y_part = nc.dram_tensor("y_part", [TT, C], f32, kind="ExternalOutput")

    xt_ap = x_t.rearrange("(ko p) t -> p ko t", p=P)  # [128, 8, TT]
    wq_ap = w_qkv_c.rearrange("(ko p) f -> p ko f", p=P)  # [128, 8, 384]

    def segs(off):
        """Split [off, 1024) at the 512 psum-bank boundary."""
        if off < 512:
            return [(off, 512), (512, 1024)]
        if off < 1024:
            return [(off, 1024)]
        return []

    with tile.TileContext(nc) as tc:
        with (
            tc.tile_pool(name="cst", bufs=1) as cst,
            tc.tile_pool(name="stage", bufs=2) as stage,
            tc.tile_pool(name="xr", bufs=2) as xrp,
            tc.tile_pool(name="qk", bufs=2) as qkp,
            tc.tile_pool(name="va", bufs=2) as vap,
            tc.tile_pool(name="yt", bufs=2) as ytp,
            tc.tile_pool(name="es", bufs=3) as esp,
            tc.tile_pool(name="oo", bufs=3) as oop,
            tc.tile_pool(name="rr", bufs=2) as rrp,
            tc.tile_pool(name="yun", bufs=2) as yun,
            tc.tile_pool(name="lt", bufs=2) as ltp,
            tc.tile_pool(name="psa", bufs=2, space="PSUM") as psa,
            tc.tile_pool(name="pss", bufs=2, space="PSUM") as pss,
            tc.tile_pool(name="psy", bufs=1, space="PSUM") as psy,
            tc.tile_pool(name="dr", bufs=2, space="DRAM") as drp,
        ):
            # ---- constants / weights ----
            ident = cst.tile([P, P], f32)
            make_identity(nc, ident[:])

            # transposed causal mask: mask[p, j] = 0 if j >= p else -1e9
            maskT = cst.tile([P, P], f32)
            nc.gpsimd.memset(maskT[:], 0.0)
            nc.gpsimd.affine_select(
                out=maskT[:],
                in_=maskT[:],
                compare_op=mybir.AluOpType.is_ge,
                fill=-1e9,
                base=0,
                pattern=[[1, P]],
                channel_multiplier=-1,
            )

            ones_f = cst.tile([1, DH], f32)
            nc.vector.memset(ones_f[:], 1.0)
            ones_r = cst.tile([1, DH], r32)
            nc.vector.tensor_copy(ones_r[:], ones_f[:])

            ones_col = cst.tile([P, T // P, 2, 1], f32)
            nc.vector.memset(ones_col[:], 1.0)

            wq_f = stage.tile([P, 8, 3 * CPC], f32, tag="stage")
            nc.sync.dma_start(wq_f[:], wq_ap[:])
            wq_r = cst.tile([P, 8, 3 * CPC], r32)
            nc.vector.tensor_copy(wq_r[:], wq_f[:])

            wp_f = stage.tile([CPC, C], f32, tag="stage")
            nc.sync.dma_start(wp_f[:], w_proj_c[:])
            wp_r = cst.tile([CPC, C], r32)
            nc.vector.tensor_copy(wp_r[:], wp_f[:])

            b_sb = cst.tile([P, 3], f32)
            nc.sync.dma_start(b_sb[:], b_qkv_c[:])

            for b in range(B):
                bt0 = b * T

                # ---- A: QKV ----
                qT = qkp.tile([P, T], r32, tag="q")
                kT = qkp.tile([P, T], r32, tag="k")
                vT = qkp.tile([P, T], f32, tag="v")
                dsts = [qT, kT, vT]
                for n in range(T // 512):
                    x_f = stage.tile([P, 8, 512], f32, tag="stage")
                    nc.sync.dma_start(
                        x_f[:], xt_ap[:, :, bt0 + n * 512 : bt0 + (n + 1) * 512]
                    )
                    x_r = xrp.tile([P, 8, 512], r32)
                    nc.vector.tensor_copy(x_r[:], x_f[:])
                    for m in range(3):
                        ps = psa.tile([P, 512], f32, tag="a")
                        for ko in range(8):
                            nc.tensor.matmul(
                                ps[:],
                                wq_r[:, ko, m * P : (m + 1) * P],
                                x_r[:, ko, :],
                                start=(ko == 0),
                                stop=(ko == 7),
                            )
                        if F_BIAS_DVE:
                            nc.vector.tensor_scalar_add(
                                dsts[m][:, n * 512 : (n + 1) * 512],
                                ps[:],
                                b_sb[:, m : m + 1],
                            )
                        else:
                            nc.scalar.activation(
                                dsts[m][:, n * 512 : (n + 1) * 512],
                                ps[:],
                                Identity,
                                bias=b_sb[:, m : m + 1],
                            )

                # ---- B: v -> token-major with ones column ----
                v_aug = vap.tile([P, T // P, 2, DH + 1], r32)
                nc.vector.tensor_copy(v_aug[:, :, :, DH : DH + 1], ones_col[:])
                for t in range(T // P):
                    pt = psa.tile([P, P], f32, tag="a")
                    nc.tensor.transpose(pt[:], vT[:, t * P : (t + 1) * P], ident[:])
                    if F_VAUG_MERGE:
                        nc.vector.tensor_copy(
                            v_aug[:, t, :, 0:DH],
                            pt[:].rearrange("p (h d) -> p h d", h=2),
                        )
                    else:
                        nc.vector.tensor_copy(v_aug[:, t, 0, 0:DH], pt[:, 0:DH])
                        nc.vector.tensor_copy(v_aug[:, t, 1, 0:DH], pt[:, DH:P])

                # ---- C: attention ----
                yT = ytp.tile([P, T], r32)
                for h in range(2):
                    h0 = h * DH
                    for qg in range(T // 1024):
                        q0 = qg * 1024
                        ktmax = 8 * qg + 8
                        ps_y = psy.tile([P, 1024], f32)

                        def emit_av(kt, es, off):
                            for c0, c1 in segs(off):
                                nc.tensor.matmul(
                                    ps_y[0 : DH + 1, c0:c1],
                                    v_aug[:, kt, h, :],
                                    es[:, c0:c1],
                                    start=(kt == 0),
                                    stop=(kt == ktmax - 1),
                                    skip_group_check=True,
                                )

                        pending = None
                        for kt in range(ktmax):
                            d = kt - 8 * qg
                            off = max(0, d * P)
                            ps_s = pss.tile([P, 1024], f32, tag="s")
                            for c0, c1 in segs(off):
                                nc.tensor.matmul(
                                    ps_s[:, c0:c1],
                                    kT[h0 : h0 + DH, kt * P : (kt + 1) * P],
                                    qT[h0 : h0 + DH, q0 + c0 : q0 + c1],
                                    start=True,
                                    stop=True,
                                )
                            if d >= 0:
                                nc.vector.tensor_tensor(
                                    ps_s[:, off : off + P],
                                    ps_s[:, off : off + P],
                                    maskT[:],
                                    ADD,
                                )
                            es = esp.tile([P, 1024], r32)
                            nc.scalar.activation(
                                es[:, off:1024], ps_s[:, off:1024], Exp, scale=0.125
                            )
                            if pending is not None:
                                emit_av(*pending)
                            pending = (kt, es, off)
                        emit_av(*pending)
                        # normalize: one copy frees psY; reciprocal runs on 128
                        # partitions via a DRAM-bounce reshape of the l row;
                        # r is broadcast across partitions with a K=1 matmul.
                        if F_EPI >= 1:
                            y_un = yun.tile([DH + 1, 1024], f32)
                            nc.vector.tensor_copy(y_un[:], ps_y[0 : DH + 1, :])
                            if F_EPI >= 3:
                                l_dram = drp.tile([1024], f32, tag="ld")
                                nc.sync.dma_start(l_dram[:], y_un[DH : DH + 1, :])
                                l_t = ltp.tile([P, 8], f32, tag="lt")
                                nc.sync.dma_start(
                                    l_t[:], l_dram.rearrange("(p f) -> p f", p=P)
                                )
                                r_t = ltp.tile([P, 8], f32, tag="rt")
                                nc.vector.reciprocal(r_t[:], l_t[:])
                                r_dram = drp.tile([1024], f32, tag="rd")
                                nc.sync.dma_start(
                                    r_dram.rearrange("(p f) -> p f", p=P), r_t[:]
                                )
                                r_f = rrp.tile([1, 1024], f32, tag="rf")
                                nc.sync.dma_start(r_f[:], r_dram[:].unsqueeze(0))
                            else:
                                r_f = rrp.tile([1, 1024], f32, tag="rf")
                                nc.vector.reciprocal(r_f[:], y_un[DH : DH + 1, :])
                            r_r = rrp.tile([1, 1024], r32, tag="rr")
                            nc.vector.tensor_copy(r_r[:], r_f[:])
                            ps_b = pss.tile([P, 1024], f32, tag="s")
                            for c0, c1 in ((0, 512), (512, 1024)):
                                nc.tensor.matmul(
                                    ps_b[0:DH, c0:c1],
                                    ones_r[:],
                                    r_r[:, c0:c1],
                                    start=True,
                                    stop=True,
                                )
                            nc.vector.tensor_tensor(
                                yT[h0 : h0 + DH, q0 : q0 + 1024],
                                y_un[0:DH, :],
                                ps_b[0:DH, :],
                                MULT,
                            )
                        else:
                            r_f = rrp.tile([1, 1024], f32, tag="rf")
                            nc.vector.reciprocal(r_f[:], ps_y[DH : DH + 1, :])
                            r_r = rrp.tile([1, 1024], r32, tag="rr")
                            nc.vector.tensor_copy(r_r[:], r_f[:])
                            ps_b = pss.tile([P, 1024], f32, tag="s")
                            for c0, c1 in ((0, 512), (512, 1024)):
                                nc.tensor.matmul(
                                    ps_b[0:DH, c0:c1],
                                    ones_r[:],
                                    r_r[:, c0:c1],
                                    start=True,
                                    stop=True,
                                )
                            sb_b = rrp.tile([DH, 1024], f32, tag="rb")
                            nc.scalar.copy(sb_b[:], ps_b[0:DH, :])
                            nc.vector.tensor_tensor(
                                yT[h0 : h0 + DH, q0 : q0 + 1024],
                                ps_y[0:DH, :],
                                sb_b[:],
                                MULT,
                            )

                # ---- D: proj ----
                for mt in range(T // P):
                    for ng in range(C // 512):
                        ps = psa.tile([P, 512], f32, tag="a")
                        nc.tensor.matmul(
                            ps[:],
                            yT[:, mt * P : (mt + 1) * P],
                            wp_r[:, ng * 512 : (ng + 1) * 512],
                            start=True,
                            stop=True,
                        )
                        o = oop.tile([P, 512], f32)
                        nc.vector.tensor_copy(o[:], ps[:])
                        nc.sync.dma_start(
                            y_part[
                                bt0 + mt * P : bt0 + (mt + 1) * P,
                                ng * 512 : (ng + 1) * 512,
                            ],
                            o[:],
                        )

    nc.finalize()
    return nc


def kernel(x, w_qkv, b_qkv, w_proj, b_proj):
    global LAST_EXEC_NS
    from concourse.bass_utils import run_bass_kernel_spmd

    x = np.asarray(x, dtype=np.float32)
    w_qkv = np.asarray(w_qkv, dtype=np.float32)
    b_qkv = np.asarray(b_qkv, dtype=np.float32)
    w_proj = np.asarray(w_proj, dtype=np.float32)
    b_proj = np.asarray(b_proj, dtype=np.float32)

    x_t = np.ascontiguousarray(x.reshape(TT, C).T)

    in_maps = []
    for c in range(NCORES):
        s = c * CPC
        wq = np.ascontiguousarray(
            np.concatenate(
                [
                    w_qkv[:, s : s + CPC],
                    w_qkv[:, C + s : C + s + CPC],
                    w_qkv[:, 2 * C + s : 2 * C + s + CPC],
                ],
                axis=1,
            )
        )
        bq = np.ascontiguousarray(
            np.stack(
                [
                    b_qkv[s : s + CPC],
                    b_qkv[C + s : C + s + CPC],
                    b_qkv[2 * C + s : 2 * C + s + CPC],
                ],
                axis=1,
            )
        )
        wp = np.ascontiguousarray(w_proj[s : s + CPC, :])
        in_maps.append(
            {"x_t": x_t, "w_qkv_c": wq, "b_qkv_c": bq, "w_proj_c": wp}
        )

    if not _NC_CACHE:
        _NC_CACHE.append(_build())
    nc = _NC_CACHE[0]

    res = run_bass_kernel_spmd(
        nc, in_maps, list(range(NCORES)), trace=TRACE
    )
    LAST_EXEC_NS = res.exec_time_ns

    out = res.results[0]["y_part"].astype(np.float64)
    for c in range(1, NCORES):
        out += res.results[c]["y_part"]
    out = (out + b_proj).astype(np.float32)
    return out.reshape(B, T, C)


# revision 12
# speedup vs baseline: 1.2561x; 1.2561x over previous
"""Causal self-attention (B=4, T=2048, C=1024, H=16, Dh=64) on 8 TRN2 cores.

Sharding: tensor-parallel over heads. Core c owns heads {2c, 2c+1}:
  - w_qkv column-sliced -> [1024, 384] (128 q-cols | 128 k-cols | 128 v-cols)
  - w_proj row-sliced   -> [128, 1024]
  - x is broadcast to all cores, pre-transposed on host to feature-major
    x_t [1024, B*T] so the contraction dim (C) lands on SBUF partitions.
  - each core emits a partial y [B*T, 1024]; host sums the 8 partials and
    adds b_proj.

Per-core pipeline (all matmuls fp32r = full-rate fp32):
  A. QKV: qkv_T = w_slice.T @ x_t  -> q,k feature-major [128, T] fp32r;
     v feature-major f32.
  B. v PE-transposed to token-major [Tk,65] tiles with a fused ones column
     (so row 64 of the att@v psum accumulates the softmax denominator l).
  C. Attention per (head, 1024-query group), key chunks of 128:
     S^T = k_chunk.T @ q (K=64 matmul), causal mask add on the diagonal
     chunk, Exp on ACT (scale=1/8, no max subtraction - scores are O(10)),
     att@v accumulates y^T[65, Tq] in PSUM. The av matmuls trail the S
     matmuls by one chunk so the in-order PE queue never waits on ACT.
     Normalization: one DVE copy frees the psum accumulator, l bounces
     through DRAM into [128, 8] for a parallel reciprocal, and the
     PE-side tail (K=1 ones-matmul broadcast of r + DVE multiply into
     y_T) is DEFERRED into the next group's kt loop so the PE stream
     never idles on the reciprocal round-trip (idle >3.4us would drop
     the PE clock from 2.4 to 1.2 GHz via HAM).
  D. Proj: y_part[tok, :] = y_T.T @ w_proj_slice, written token-major.
"""

import sys

sys.path.insert(0, "/opt/trn_rl_repo")
import numpy as np

B = 4
T = 2048
C = 1024
H = 16
DH = 64
NCORES = 8
CPC = 128  # channels (=2 heads * 64) per core
TT = B * T
P = 128

TRACE = False
LAST_EXEC_NS = None

_NC_CACHE = []


def _build():
    import concourse.bacc as bacc
    import concourse.mybir as mybir
    import concourse.tile as tile
    from concourse.masks import make_identity

    f32 = mybir.dt.float32
    r32 = mybir.dt.float32r
    Exp = mybir.ActivationFunctionType.Exp
    ADD = mybir.AluOpType.add
    MULT = mybir.AluOpType.mult

    nc = bacc.Bacc(target_bir_lowering=False)

    x_t = nc.dram_tensor("x_t", [C, TT], f32, kind="ExternalInput")
    w_qkv_c = nc.dram_tensor("w_qkv_c", [C, 3 * CPC], f32, kind="ExternalInput")
    b_qkv_c = nc.dram_tensor("b_qkv_c", [P, 3], f32, kind="ExternalInput")
    w_proj_c = nc.dram_tensor("w_proj_c", [CPC, C], f32, kind="ExternalInput")
    y_part = nc.dram_tensor("y_part", [TT, C], f32, kind="ExternalOutput")

    xt_ap = x_t.rearrange("(ko p) t -> p ko t", p=P)  # [128, 8, TT]
    wq_ap = w_qkv_c.rearrange("(ko p) f -> p ko f", p=P)  # [128, 8, 384]

    def segs(off):
        """Split [off, 1024) at the 512 psum-bank boundary."""
        if off < 512:
            return [(off, 512), (512, 1024)]
        if off < 1024:
            return [(off, 1024)]
        return []

    with tile.TileContext(nc) as tc:
        with (
            tc.tile_pool(name="cst", bufs=1) as cst,
            tc.tile_pool(name="stage", bufs=2) as stage,
            tc.tile_pool(name="xr", bufs=2) as xrp,
            tc.tile_pool(name="qk", bufs=2) as qkp,
            tc.tile_pool(name="va", bufs=2) as vap,
            tc.tile_pool(name="yt", bufs=2) as ytp,
            tc.tile_pool(name="es", bufs=3) as esp,
            tc.tile_pool(name="oo", bufs=3) as oop,
            tc.tile_pool(name="rr", bufs=2) as rrp,
            tc.tile_pool(name="yun", bufs=2) as yun,
            tc.tile_pool(name="lt", bufs=2) as ltp,
            tc.tile_pool(name="psa", bufs=2, space="PSUM") as psa,
            tc.tile_pool(name="pss", bufs=2, space="PSUM") as pss,
            tc.tile_pool(name="psy", bufs=1, space="PSUM") as psy,
            tc.tile_pool(name="dr", bufs=2, space="DRAM") as drp,
        ):
            # ---- constants / weights ----
            ident = cst.tile([P, P], f32)
            make_identity(nc, ident[:])

            # transposed causal mask: mask[p, j] = 0 if j >= p else -1e9
            maskT = cst.tile([P, P], f32)
            nc.gpsimd.memset(maskT[:], 0.0)
            nc.gpsimd.affine_select(
                out=maskT[:],
                in_=maskT[:],
                compare_op=mybir.AluOpType.is_ge,
                fill=-1e9,
                base=0,
                pattern=[[1, P]],
                channel_multiplier=-1,
            )

            ones_f = cst.tile([1, DH], f32)
            nc.vector.memset(ones_f[:], 1.0)
            ones_r = cst.tile([1, DH], r32)
            nc.vector.tensor_copy(ones_r[:], ones_f[:])

            ones_col = cst.tile([P, T // P, 2, 1], f32)
            nc.vector.memset(ones_col[:], 1.0)

            wq_f = stage.tile([P, 8, 3 * CPC], f32, tag="stage")
            nc.sync.dma_start(wq_f[:], wq_ap[:])
            wq_r = cst.tile([P, 8, 3 * CPC], r32)
            nc.vector.tensor_copy(wq_r[:], wq_f[:])

            wp_f = stage.tile([CPC, C], f32, tag="stage")
            nc.sync.dma_start(wp_f[:], w_proj_c[:])
            wp_r = cst.tile([CPC, C], r32)
            nc.vector.tensor_copy(wp_r[:], wp_f[:])

            b_sb = cst.tile([P, 3], f32)
            nc.sync.dma_start(b_sb[:], b_qkv_c[:])

            # deferred PE-side epilogue tail of the previous attention group
            pending_late = [None]

            def pump_late():
                if pending_late[0] is not None:
                    pending_late[0]()
                    pending_late[0] = None

            for b in range(B):
                bt0 = b * T

                # ---- A: QKV ----
                qT = qkp.tile([P, T], r32, tag="q")
                kT = qkp.tile([P, T], r32, tag="k")
                vT = qkp.tile([P, T], f32, tag="v")
                dsts = [qT, kT, vT]
                for n in range(T // 512):
                    x_f = stage.tile([P, 8, 512], f32, tag="stage")
                    nc.sync.dma_start(
                        x_f[:], xt_ap[:, :, bt0 + n * 512 : bt0 + (n + 1) * 512]
                    )
                    x_r = xrp.tile([P, 8, 512], r32)
                    nc.vector.tensor_copy(x_r[:], x_f[:])
                    for m in range(3):
                        ps = psa.tile([P, 512], f32, tag="a")
                        for ko in range(8):
                            nc.tensor.matmul(
                                ps[:],
                                wq_r[:, ko, m * P : (m + 1) * P],
                                x_r[:, ko, :],
                                start=(ko == 0),
                                stop=(ko == 7),
                            )
                        nc.vector.tensor_scalar_add(
                            dsts[m][:, n * 512 : (n + 1) * 512],
                            ps[:],
                            b_sb[:, m : m + 1],
                        )

                # ---- B: v -> token-major with ones column ----
                v_aug = vap.tile([P, T // P, 2, DH + 1], r32)
                nc.vector.tensor_copy(v_aug[:, :, :, DH : DH + 1], ones_col[:])
                for t in range(T // P):
                    pt = psa.tile([P, P], f32, tag="a")
                    nc.tensor.transpose(pt[:], vT[:, t * P : (t + 1) * P], ident[:])
                    nc.vector.tensor_copy(
                        v_aug[:, t, :, 0:DH],
                        pt[:].rearrange("p (h d) -> p h d", h=2),
                    )

                # ---- C: attention ----
                yT = ytp.tile([P, T], r32)
                for h in range(2):
                    h0 = h * DH
                    for qg in range(T // 1024):
                        q0 = qg * 1024
                        ktmax = 8 * qg + 8
                        ps_y = psy.tile([P, 1024], f32)

                        def emit_av(kt, es, off, ps_y=ps_y, h=h, ktmax=ktmax):
                            for c0, c1 in segs(off):
                                nc.tensor.matmul(
                                    ps_y[0 : DH + 1, c0:c1],
                                    v_aug[:, kt, h, :],
                                    es[:, c0:c1],
                                    start=(kt == 0),
                                    stop=(kt == ktmax - 1),
                                    skip_group_check=True,
                                )

                        pending_av = None
                        for kt in range(ktmax):
                            d = kt - 8 * qg
                            off = max(0, d * P)
                            ps_s = pss.tile([P, 1024], f32, tag="s")
                            for c0, c1 in segs(off):
                                nc.tensor.matmul(
                                    ps_s[:, c0:c1],
                                    kT[h0 : h0 + DH, kt * P : (kt + 1) * P],
                                    qT[h0 : h0 + DH, q0 + c0 : q0 + c1],
                                    start=True,
                                    stop=True,
                                )
                            if d >= 0:
                                nc.vector.tensor_tensor(
                                    ps_s[:, off : off + P],
                                    ps_s[:, off : off + P],
                                    maskT[:],
                                    ADD,
                                )
                            es = esp.tile([P, 1024], r32)
                            nc.scalar.activation(
                                es[:, off:1024], ps_s[:, off:1024], Exp, scale=0.125
                            )
                            if pending_av is not None:
                                emit_av(*pending_av)
                            pending_av = (kt, es, off)
                            if kt == 3:
                                pump_late()
                        emit_av(*pending_av)

                        # early epilogue: free psY, compute r = 1/l via a
                        # DRAM-bounce reshape (reciprocal on 128 partitions)
                        y_un = yun.tile([DH + 1, 1024], f32)
                        nc.vector.tensor_copy(y_un[:], ps_y[0 : DH + 1, :])
                        l_dram = drp.tile([1024], f32, tag="ld")
                        nc.sync.dma_start(l_dram[:], y_un[DH : DH + 1, :])
                        l_t = ltp.tile([P, 8], f32, tag="lt")
                        nc.sync.dma_start(
                            l_t[:], l_dram.rearrange("(p f) -> p f", p=P)
                        )
                        r_t = ltp.tile([P, 8], f32, tag="rt")
                        nc.vector.reciprocal(r_t[:], l_t[:])
                        r_dram = drp.tile([1024], f32, tag="rd")
                        nc.sync.dma_start(
                            r_dram.rearrange("(p f) -> p f", p=P), r_t[:]
                        )
                        r_f = rrp.tile([1, 1024], f32, tag="rf")
                        nc.sync.dma_start(r_f[:], r_dram[:].unsqueeze(0))
                        r_r = rrp.tile([1, 1024], r32, tag="rr")
                        nc.vector.tensor_copy(r_r[:], r_f[:])

                        def late(y_un=y_un, r_r=r_r, h0=h0, q0=q0, yT=yT):
                            for half in (0, 1):
                                c0 = half * 512
                                ps_b = psa.tile([P, 512], f32, tag="a")
                                nc.tensor.matmul(
                                    ps_b[0:DH, :],
                                    ones_r[:],
                                    r_r[:, c0 : c0 + 512],
                                    start=True,
                                    stop=True,
                                )
                                nc.vector.tensor_tensor(
                                    yT[h0 : h0 + DH, q0 + c0 : q0 + c0 + 512],
                                    y_un[0:DH, c0 : c0 + 512],
                                    ps_b[0:DH, :],
                                    MULT,
                                )

                        pending_late[0] = late

                # ---- D: proj ----
                for mt in range(T // P):
                    for ng in range(C // 512):
                        ps = psa.tile([P, 512], f32, tag="a")
                        nc.tensor.matmul(
                            ps[:],
                            yT[:, mt * P : (mt + 1) * P],
                            wp_r[:, ng * 512 : (ng + 1) * 512],
                            start=True,
                            stop=True,
                        )
                        o = oop.tile([P, 512], f32)
                        nc.vector.tensor_copy(o[:], ps[:])
                        nc.sync.dma_start(
                            y_part[
                                bt0 + mt * P : bt0 + (mt + 1) * P,
                                ng * 512 : (ng + 1) * 512,
                            ],
                            o[:],
                        )
                    if mt == 1:
                        pump_late()

            pump_late()

    nc.finalize()
    return nc


def kernel(x, w_qkv, b_qkv, w_proj, b_proj):
    global LAST_EXEC_NS
    from concourse.bass_utils import run_bass_kernel_spmd

    x = np.asarray(x, dtype=np.float32)
    w_qkv = np.asarray(w_qkv, dtype=np.float32)
    b_qkv = np.asarray(b_qkv, dtype=np.float32)
    w_proj = np.asarray(w_proj, dtype=np.float32)
    b_proj = np.asarray(b_proj, dtype=np.float32)

    x_t = np.ascontiguousarray(x.reshape(TT, C).T)

    in_maps = []
    for c in range(NCORES):
        s = c * CPC
        wq = np.ascontiguousarray(
            np.concatenate(
                [
                    w_qkv[:, s : s + CPC],
                    w_qkv[:, C + s : C + s + CPC],
                    w_qkv[:, 2 * C + s : 2 * C + s + CPC],
                ],
                axis=1,
            )
        )
        bq = np.ascontiguousarray(
            np.stack(
                [
                    b_qkv[s : s + CPC],
                    b_qkv[C + s : C + s + CPC],
                    b_qkv[2 * C + s : 2 * C + s + CPC],
                ],
                axis=1,
            )
        )
        wp = np.ascontiguousarray(w_proj[s : s + CPC, :])
        in_maps.append(
            {"x_t": x_t, "w_qkv_c": wq, "b_qkv_c": bq, "w_proj_c": wp}
        )

    if not _NC_CACHE:
        _NC_CACHE.append(_build())
    nc = _NC_CACHE[0]

    res = run_bass_kernel_spmd(
        nc, in_maps, list(range(NCORES)), trace=TRACE
    )
    LAST_EXEC_NS = res.exec_time_ns

    out = res.results[0]["y_part"].astype(np.float64)
    for c in range(1, NCORES):
        out += res.results[c]["y_part"]
    out = (out + b_proj).astype(np.float32)
    return out.reshape(B, T, C)


# revision 14
# speedup vs baseline: 1.3605x; 1.0831x over previous
"""Causal self-attention (B=4, T=2048, C=1024, H=16, Dh=64) on 8 TRN2 cores.

Sharding: tensor-parallel over heads. Core c owns heads {2c, 2c+1}:
  - w_qkv column-sliced -> [1024, 384] (128 q-cols | 128 k-cols | 128 v-cols)
  - w_proj row-sliced   -> [128, 1024]
  - x is broadcast to all cores, pre-transposed on host to feature-major
    x_t [1024, B*T] so the contraction dim (C) lands on SBUF partitions.
  - each core emits a partial y [B*T, 1024]; host sums the 8 partials and
    adds b_proj.

Per-core pipeline (all matmuls fp32r = full-rate fp32):
  A. QKV: qkv_T = w_slice.T @ x_t -> q,k feature-major [128, T] fp32r;
     v feature-major f32, then PE-transposed to token-major [Tk, 65] with a
     fused ones column (row 64 of the att@v psum = softmax denominator l).
  B. Attention per (head, 1024-query group), key chunks of 128:
     S^T = k_chunk.T @ q (K=64), causal mask add on the diagonal chunk,
     Exp on ACT (scale=1/8, no max subtraction - scores are O(10)),
     att@v accumulates y^T[65, Tq] in PSUM.
  C. Proj: y_part[tok, :] = y_T.T @ w_proj_slice, written token-major.

Scheduling: the TRN2 engines execute their queues IN ORDER, and the PE
clock halves (HAM) whenever the PE idles >~1-3us, so the emission order is
software-pipelined to keep the PE stream dense:
  - av matmuls trail the S matmuls by one key chunk (never wait on ACT);
  - next-batch QKV psum-groups are interleaved as PE filler at each
    attention group start (covers the S->mask->exp->av chain latency);
  - softmax normalization is split: the psum accumulator is freed with one
    DVE copy, l bounces through DRAM into [128, 8] for a parallel
    reciprocal, and the PE-side tail (K=1 ones-matmul broadcast of r,
    DVE multiply into y_T) is deferred ~6 chunks into the next group.
"""

import sys

sys.path.insert(0, "/opt/trn_rl_repo")
import numpy as np

B = 4
T = 2048
C = 1024
H = 16
DH = 64
NCORES = 8
CPC = 128  # channels (=2 heads * 64) per core
TT = B * T
P = 128

TRACE = False
LAST_EXEC_NS = None

_NC_CACHE = []


def _build():
    import concourse.bacc as bacc
    import concourse.mybir as mybir
    import concourse.tile as tile
    from concourse.masks import make_identity

    f32 = mybir.dt.float32
    r32 = mybir.dt.float32r
    Exp = mybir.ActivationFunctionType.Exp
    ADD = mybir.AluOpType.add
    MULT = mybir.AluOpType.mult

    nc = bacc.Bacc(target_bir_lowering=False)

    x_t = nc.dram_tensor("x_t", [C, TT], f32, kind="ExternalInput")
    w_qkv_c = nc.dram_tensor("w_qkv_c", [C, 3 * CPC], f32, kind="ExternalInput")
    b_qkv_c = nc.dram_tensor("b_qkv_c", [P, 3], f32, kind="ExternalInput")
    w_proj_c = nc.dram_tensor("w_proj_c", [CPC, C], f32, kind="ExternalInput")
    y_part = nc.dram_tensor("y_part", [TT, C], f32, kind="ExternalOutput")

    xt_ap = x_t.rearrange("(ko p) t -> p ko t", p=P)  # [128, 8, TT]
    wq_ap = w_qkv_c.rearrange("(ko p) f -> p ko f", p=P)  # [128, 8, 384]

    def segs(off):
        """Split [off, 1024) at the 512 psum-bank boundary."""
        if off < 512:
            return [(off, 512), (512, 1024)]
        if off < 1024:
            return [(off, 1024)]
        return []

    with tile.TileContext(nc) as tc:
        with (
            tc.tile_pool(name="cst", bufs=1) as cst,
            tc.tile_pool(name="stage", bufs=2) as stage,
            tc.tile_pool(name="xr", bufs=2) as xrp,
            tc.tile_pool(name="qk", bufs=2) as qkp,
            tc.tile_pool(name="va", bufs=2) as vap,
            tc.tile_pool(name="yt", bufs=2) as ytp,
            tc.tile_pool(name="es", bufs=3) as esp,
            tc.tile_pool(name="oo", bufs=3) as oop,
            tc.tile_pool(name="rr", bufs=2) as rrp,
            tc.tile_pool(name="yun", bufs=2) as yun,
            tc.tile_pool(name="lt", bufs=2) as ltp,
            tc.tile_pool(name="psa", bufs=2, space="PSUM") as psa,
            tc.tile_pool(name="pss", bufs=2, space="PSUM") as pss,
            tc.tile_pool(name="psy", bufs=1, space="PSUM") as psy,
            tc.tile_pool(name="dr", bufs=2, space="DRAM") as drp,
        ):
            # ---- constants / weights ----
            ident = cst.tile([P, P], f32)
            make_identity(nc, ident[:])

            # transposed causal mask: mask[p, j] = 0 if j >= p else -1e9
            maskT = cst.tile([P, P], f32)
            nc.gpsimd.memset(maskT[:], 0.0)
            nc.gpsimd.affine_select(
                out=maskT[:],
                in_=maskT[:],
                compare_op=mybir.AluOpType.is_ge,
                fill=-1e9,
                base=0,
                pattern=[[1, P]],
                channel_multiplier=-1,
            )

            ones_f = cst.tile([1, DH], f32)
            nc.vector.memset(ones_f[:], 1.0)
            ones_r = cst.tile([1, DH], r32)
            nc.vector.tensor_copy(ones_r[:], ones_f[:])

            ones_col = cst.tile([P, T // P, 2, 1], f32)
            nc.vector.memset(ones_col[:], 1.0)

            wq_f = stage.tile([P, 8, 3 * CPC], f32, tag="stage")
            nc.sync.dma_start(wq_f[:], wq_ap[:])
            wq_r = cst.tile([P, 8, 3 * CPC], r32)
            nc.vector.tensor_copy(wq_r[:], wq_f[:])

            wp_f = stage.tile([CPC, C], f32, tag="stage")
            nc.sync.dma_start(wp_f[:], w_proj_c[:])
            wp_r = cst.tile([CPC, C], r32)
            nc.vector.tensor_copy(wp_r[:], wp_f[:])

            b_sb = cst.tile([P, 3], f32)
            nc.sync.dma_start(b_sb[:], b_qkv_c[:])

            # ---- per-batch QKV + v-transpose as a resumable step stream ----
            states = {}

            def make_state(b):
                st = {
                    "qT": qkp.tile([P, T], r32, tag="q", name="qT"),
                    "kT": qkp.tile([P, T], r32, tag="k", name="kT"),
                    "vT": qkp.tile([P, T], f32, tag="v", name="vT"),
                    "yT": ytp.tile([P, T], r32, name="yT"),
                }
                bt0 = b * T
                dsts = [st["qT"], st["kT"], st["vT"]]

                def gen():
                    for n in range(T // 512):
                        x_f = stage.tile([P, 8, 512], f32, tag="stage")
                        nc.sync.dma_start(
                            x_f[:],
                            xt_ap[:, :, bt0 + n * 512 : bt0 + (n + 1) * 512],
                        )
                        x_r = xrp.tile([P, 8, 512], r32)
                        nc.vector.tensor_copy(x_r[:], x_f[:])
                        for m in range(3):
                            ps = psa.tile([P, 512], f32, tag="a")
                            for ko in range(8):
                                nc.tensor.matmul(
                                    ps[:],
                                    wq_r[:, ko, m * P : (m + 1) * P],
                                    x_r[:, ko, :],
                                    start=(ko == 0),
                                    stop=(ko == 7),
                                )
                            nc.vector.tensor_scalar_add(
                                dsts[m][:, n * 512 : (n + 1) * 512],
                                ps[:],
                                b_sb[:, m : m + 1],
                            )
                            yield

                    v_aug = vap.tile([P, T // P, 2, DH + 1], r32)
                    st["v_aug"] = v_aug
                    nc.vector.tensor_copy(
                        v_aug[:, :, :, DH : DH + 1], ones_col[:]
                    )
                    for t in range(T // P):
                        pt = psa.tile([P, P], f32, tag="a")
                        nc.tensor.transpose(
                            pt[:], st["vT"][:, t * P : (t + 1) * P], ident[:]
                        )
                        nc.vector.tensor_copy(
                            v_aug[:, t, :, 0:DH],
                            pt[:].rearrange("p (h d) -> p h d", h=2),
                        )
                        yield

                st["gen"] = gen()
                return st

            def get_state(b):
                if b not in states:
                    states[b] = make_state(b)
                return states[b]

            def filler(b):
                """Emit one next-batch QKV/transpose step as PE filler."""
                if b < B:
                    next(get_state(b)["gen"], None)

            # deferred PE-side epilogue tail of the previous attention group
            pending_late = [None]

            def pump_late():
                if pending_late[0] is not None:
                    pending_late[0]()
                    pending_late[0] = None

            for b in range(B):
                bt0 = b * T
                st = get_state(b)
                for _ in st["gen"]:
                    pass
                qT, kT, v_aug, yT = st["qT"], st["kT"], st["v_aug"], st["yT"]

                # ---- attention ----
                for h in range(2):
                    h0 = h * DH
                    for qg in range(T // 1024):
                        q0 = qg * 1024
                        ktmax = 8 * qg + 8
                        ps_y = psy.tile([P, 1024], f32)

                        def emit_av(kt, es, off, ps_y=ps_y, h=h, ktmax=ktmax,
                                    v_aug=v_aug):
                            for c0, c1 in segs(off):
                                nc.tensor.matmul(
                                    ps_y[0 : DH + 1, c0:c1],
                                    v_aug[:, kt, h, :],
                                    es[:, c0:c1],
                                    start=(kt == 0),
                                    stop=(kt == ktmax - 1),
                                    skip_group_check=True,
                                )

                        pending_av = None
                        for kt in range(ktmax):
                            d = kt - 8 * qg
                            off = max(0, d * P)
                            ps_s = pss.tile([P, 1024], f32, tag="s")
                            for c0, c1 in segs(off):
                                nc.tensor.matmul(
                                    ps_s[:, c0:c1],
                                    kT[h0 : h0 + DH, kt * P : (kt + 1) * P],
                                    qT[h0 : h0 + DH, q0 + c0 : q0 + c1],
                                    start=True,
                                    stop=True,
                                )
                            if d >= 0:
                                nc.vector.tensor_tensor(
                                    ps_s[:, off : off + P],
                                    ps_s[:, off : off + P],
                                    maskT[:],
                                    ADD,
                                )
                            es = esp.tile([P, 1024], r32)
                            nc.scalar.activation(
                                es[:, off:1024], ps_s[:, off:1024], Exp,
                                scale=0.125,
                            )
                            if kt in (1, 2):
                                filler(b + 1)
                            if pending_av is not None:
                                emit_av(*pending_av)
                            pending_av = (kt, es, off)
                            if kt == 5:
                                pump_late()
                        emit_av(*pending_av)

                        # early epilogue: free psY, compute r = 1/l via a
                        # DRAM-bounce reshape (reciprocal on 128 partitions)
                        y_un = yun.tile([DH + 1, 1024], f32)
                        nc.vector.tensor_copy(y_un[:], ps_y[0 : DH + 1, :])
                        l_dram = drp.tile([1024], f32, tag="ld")
                        nc.sync.dma_start(l_dram[:], y_un[DH : DH + 1, :])
                        l_t = ltp.tile([P, 8], f32, tag="lt")
                        nc.sync.dma_start(
                            l_t[:], l_dram.rearrange("(p f) -> p f", p=P)
                        )
                        r_t = ltp.tile([P, 8], f32, tag="rt")
                        nc.vector.reciprocal(r_t[:], l_t[:])
                        r_dram = drp.tile([1024], f32, tag="rd")
                        nc.sync.dma_start(
                            r_dram.rearrange("(p f) -> p f", p=P), r_t[:]
                        )
                        r_f = rrp.tile([1, 1024], f32, tag="rf")
                        nc.sync.dma_start(r_f[:], r_dram[:].unsqueeze(0))
                        r_r = rrp.tile([1, 1024], r32, tag="rr")
                        nc.vector.tensor_copy(r_r[:], r_f[:])

                        def late(y_un=y_un, r_r=r_r, h0=h0, q0=q0, yT=yT):
                            for half in (0, 1):
                                c0 = half * 512
                                ps_b = psa.tile([P, 512], f32, tag="a")
                                nc.tensor.matmul(
                                    ps_b[0:DH, :],
                                    ones_r[:],
                                    r_r[:, c0 : c0 + 512],
                                    start=True,
                                    stop=True,
                                )
                                nc.vector.tensor_tensor(
                                    yT[h0 : h0 + DH, q0 + c0 : q0 + c0 + 512],
                                    y_un[0:DH, c0 : c0 + 512],
                                    ps_b[0:DH, :],
                                    MULT,
                                )

                        pending_late[0] = late

                # ---- proj ----
                for mt in range(T // P):
                    for ng in range(C // 512):
                        ps = psa.tile([P, 512], f32, tag="a")
                        nc.tensor.matmul(
                            ps[:],
                            yT[:, mt * P : (mt + 1) * P],
                            wp_r[:, ng * 512 : (ng + 1) * 512],
                            start=True,
                            stop=True,
                        )
                        o = oop.tile([P, 512], f32)
                        nc.vector.tensor_copy(o[:], ps[:])
                        nc.sync.dma_start(
                            y_part[
                                bt0 + mt * P : bt0 + (mt + 1) * P,
                                ng * 512 : (ng + 1) * 512,
                            ],
                            o[:],
                        )
                    if mt == 7:
                        pump_late()

            pump_late()

    nc.finalize()
    return nc


def kernel(x, w_qkv, b_qkv, w_proj, b_proj):
    global LAST_EXEC_NS
    from concourse.bass_utils import run_bass_kernel_spmd

    x = np.asarray(x, dtype=np.float32)
    w_qkv = np.asarray(w_qkv, dtype=np.float32)
    b_qkv = np.asarray(b_qkv, dtype=np.float32)
    w_proj = np.asarray(w_proj, dtype=np.float32)
    b_proj = np.asarray(b_proj, dtype=np.float32)

    x_t = np.ascontiguousarray(x.reshape(TT, C).T)

    in_maps = []
    for c in range(NCORES):
        s = c * CPC
        wq = np.ascontiguousarray(
            np.concatenate(
                [
                    w_qkv[:, s : s + CPC],
                    w_qkv[:, C + s : C + s + CPC],
                    w_qkv[:, 2 * C + s : 2 * C + s + CPC],
                ],
                axis=1,
            )
        )
        bq = np.ascontiguousarray(
            np.stack(
                [
                    b_qkv[s : s + CPC],
                    b_qkv[C + s : C + s + CPC],
                    b_qkv[2 * C + s : 2 * C + s + CPC],
                ],
                axis=1,
            )
        )
        wp = np.ascontiguousarray(w_proj[s : s + CPC, :])
        in_maps.append(
            {"x_t": x_t, "w_qkv_c": wq, "b_qkv_c": bq, "w_proj_c": wp}
        )

    if not _NC_CACHE:
        _NC_CACHE.append(_build())
    nc = _NC_CACHE[0]

    res = run_bass_kernel_spmd(
        nc, in_maps, list(range(NCORES)), trace=TRACE
    )
    LAST_EXEC_NS = res.exec_time_ns

    out = res.results[0]["y_part"].astype(np.float64)
    for c in range(1, NCORES):
        out += res.results[c]["y_part"]
    out = (out + b_proj).astype(np.float32)
    return out.reshape(B, T, C)
